# revision 2
# baseline (speedup 1.0000x reference)
"""AnyNet stereo-depth network on 8 Trainium2 NeuronCores.

Sharding: pure data parallelism over the batch dim B=8 -> one batch element
per NeuronCore; the tiny 3D-conv weights are replicated to every core.

kernel(**inputs) takes the FULL unsharded inputs (numpy), distributes batch
shards to the 8 cores, runs the forward pass on-device (compiled once,
cached), and gathers the full-shape outputs.
"""

import functools

import numpy as np
import jax
import jax.numpy as jnp

# ---- problem constants (hardcoded; kernel.py must be self-contained) ----
B, C = 8, 8
IMG_H, IMG_W = 512, 1280
MAXDISP = (12, 3, 3)
CH3D = (16, 4, 4)
LAYERS_3D = 4
BASELINE, FOCAL = 0.2, 320.0
MIN_DEPTH, MAX_DEPTH = 0.1, 100.0
N_CORES = 8


def _conv3d(x, w):
    # x: [B,Cin,D,H,W], w: [Cout,Cin,3,3,3], stride 1, SAME padding
    return jax.lax.conv_general_dilated(
        x, w, (1, 1, 1), 'SAME',
        dimension_numbers=('NCDHW', 'OIDHW', 'NCDHW'))


def post_3dconv(cost, w_in, w_mid, w_out):
    x = jax.nn.relu(_conv3d(cost, w_in))
    for k in range(w_mid.shape[0]):
        x = jax.nn.relu(_conv3d(x, w_mid[k]))
    return _conv3d(x, w_out)


def build_volume_2d(fl, fr, maxdisp):
    Bn, Cn, H, W = fl.shape
    d = jnp.arange(maxdisp)
    src = jnp.arange(W)[None, :] - d[:, None]
    valid = (src >= 0).astype(fl.dtype)
    fr_s = jnp.take(fr, jnp.clip(src, 0, W - 1), axis=3)
    fr_s = fr_s * valid[None, None, None]
    cost = jnp.abs(fl[:, :, :, None, :] - fr_s).sum(1)
    return jnp.transpose(cost, (0, 2, 1, 3))


def build_volume_2d3(fl, fr, maxdisp, disp):
    Bn, Cn, H, W = fl.shape
    shifts = jnp.arange(-maxdisp + 1, maxdisp, dtype=fl.dtype)
    sd = disp[:, 0][:, None] - shifts[None, :, None, None]
    xs = jnp.arange(W, dtype=fl.dtype)[None, None, None, :] - sd
    x0 = jnp.floor(xs)
    w1 = xs - x0
    x0i = x0.astype(jnp.int32)

    def sample(xi):
        inb = ((xi >= 0) & (xi <= W - 1)).astype(fl.dtype)
        idx = jnp.clip(xi, 0, W - 1)[:, :, None]
        v = jnp.take_along_axis(fr[:, None], idx, axis=-1)
        return v * inb[:, :, None]

    v = sample(x0i) * (1.0 - w1)[:, :, None] + sample(x0i + 1) * w1[:, :, None]
    return jnp.abs(fl[:, None] - v).sum(2)


def disp_reg(cost, start, end):
    p = jax.nn.softmax(-cost, axis=1)
    d = jnp.arange(start, end, dtype=cost.dtype)[None, :, None, None]
    return jnp.sum(p * d, axis=1, keepdims=True)


def upsample(x, h, w):
    Bn, Cn = x.shape[0], x.shape[1]
    return jax.image.resize(x, (Bn, Cn, h, w), method='bilinear')


def _forward(feats_l, feats_r, params, img_h, img_w):
    pred = []
    for s in range(3):
        fl, fr = feats_l[s], feats_r[s]
        H, W = fl.shape[2], fl.shape[3]
        if s == 0:
            cost = build_volume_2d(fl, fr, MAXDISP[0])
        else:
            wflow = upsample(pred[-1], H, W) * (float(H) / img_h)
            cost = build_volume_2d3(fl, fr, MAXDISP[s], wflow)
        cost = post_3dconv(cost[:, None], *params[s])[:, 0]
        if s == 0:
            d = disp_reg(cost, 0, MAXDISP[0])
        else:
            d = disp_reg(cost, -MAXDISP[s] + 1, MAXDISP[s])
        d = d * (img_h / float(H))
        d_up = upsample(d, int(img_h), int(img_w))
        pred.append(d_up if s == 0 else d_up + pred[-1])
    disp = jnp.abs(pred[0])
    depth = jnp.nan_to_num(BASELINE * FOCAL / disp,
                           nan=MAX_DEPTH, posinf=MAX_DEPTH, neginf=MIN_DEPTH)
    depth = jnp.clip(depth, MIN_DEPTH, MAX_DEPTH)
    return pred[0], pred[1], pred[2], depth


@functools.partial(jax.pmap, axis_name='b',
                   in_axes=(0, 0, 0, 0, 0, 0, None, None, None, None, None,
                            None, None, None, None),
                   static_broadcasted_argnums=())
def _pmapped(f_l0, f_r0, f_l1, f_r1, f_l2, f_r2,
             w_in0, w_mid0, w_out0,
             w_in1, w_mid1, w_out1,
             w_in2, w_mid2, w_out2):
    feats_l = (f_l0, f_l1, f_l2)
    feats_r = (f_r0, f_r1, f_r2)
    params = ((w_in0, w_mid0, w_out0),
              (w_in1, w_mid1, w_out1),
              (w_in2, w_mid2, w_out2))
    return _forward(feats_l, feats_r, params, float(IMG_H), float(IMG_W))


def kernel(f_l0, f_r0, f_l1, f_r1, f_l2, f_r2,
           w_in0, w_mid0, w_out0,
           w_in1, w_mid1, w_out1,
           w_in2, w_mid2, w_out2,
           img_h, img_w):
    """Full unsharded inputs in; full unsharded outputs out.

    Shards the batch dim across the 8 NeuronCores (1 element per core),
    replicates the conv weights, runs the compiled per-core program, and
    concatenates the per-core outputs back to full shape.
    """
    assert int(img_h) == IMG_H and int(img_w) == IMG_W

    # [B, ...] -> [n_cores, B/n_cores, ...]  (B == n_cores -> 1 each)
    per = B // N_CORES

    def shard(x):
        x = np.asarray(x, dtype=np.float32)
        return x.reshape((N_CORES, per) + x.shape[1:])

    args = [shard(f_l0), shard(f_r0), shard(f_l1), shard(f_r1),
            shard(f_l2), shard(f_r2)]
    weights = [np.asarray(w, dtype=np.float32) for w in
               (w_in0, w_mid0, w_out0, w_in1, w_mid1, w_out1,
                w_in2, w_mid2, w_out2)]

    outs = _pmapped(*args, *weights)
    # gather: [n_cores, per, 1, H, W] -> [B, 1, H, W]
    outs = [np.asarray(o).reshape((B,) + o.shape[2:]).astype(np.float32)
            for o in outs]
    return tuple(outs)


if __name__ == '__main__':
    # smoke test with random data
    rng = np.random.RandomState(0)
    inp = {
        'f_l0': rng.randn(B, C, 32, 80).astype(np.float32),
        'f_r0': rng.randn(B, C, 32, 80).astype(np.float32),
        'f_l1': rng.randn(B, C, 64, 160).astype(np.float32),
        'f_r1': rng.randn(B, C, 64, 160).astype(np.float32),
        'f_l2': rng.randn(B, C, 128, 320).astype(np.float32),
        'f_r2': rng.randn(B, C, 128, 320).astype(np.float32),
        'img_h': np.int64(IMG_H), 'img_w': np.int64(IMG_W),
    }
    for s, ch in enumerate(CH3D):
        inp[f'w_in{s}'] = rng.randn(ch, 1, 3, 3, 3).astype(np.float32) * 0.1
        inp[f'w_mid{s}'] = rng.randn(LAYERS_3D, ch, ch, 3, 3, 3).astype(np.float32) * 0.1
        inp[f'w_out{s}'] = rng.randn(1, ch, 3, 3, 3).astype(np.float32) * 0.1
    outs = kernel(**inp)
    for o in outs:
        print(o.shape, o.dtype, float(np.abs(o).mean()))
